# revision 30
# baseline (speedup 1.0000x reference)
"""EntropyProfileLoss Trainium2 kernel.

Math: for a window t of length k, sum(softmax(t)*log_softmax(t))
      = S2/S1 - ln(S1),  S1 = sum(exp(t)), S2 = sum(t*exp(t)).
Window sums for every k come from per-segment prefix scans P of
exp(x) and x*exp(x): S_k[f] = P[f+k] - P[f].

Sharding: pure data parallel over batch B=64 -> 8 cores x 8 batches.
Each core returns per-partition per-k partial L1 sums [128, 6]; the
host reduces across cores/partitions and applies the 1/(B*C*W_k)
mean scaling.

Per-core layout: 8 batches x 2 ch = 16 rows of L=2048.
Partition p = chunk*16 + row with 8 chunks of 256 window-starts per
row; each chunk carries 127 halo cols so windows up to k=128 stay
within the partition. The free dim holds [x | t] in X and 4 scan
segments [E_x | XE_x | E_t | XE_t] in EX, each segment prefixed by a
zero column (scan seed / S_k base).

Chunk 7 has only 256 real cols; its tail is padded with +30. Inside
one segment the pad terms e^30 dominate every real partial sum by
more than 2^24, so any window that touches a pad column yields
bit-identical S1/S2 (and thus negentropy) for x and t: invalid
window starts contribute exactly 0 to sum(|dx|), and no masking of
the accumulation is needed. Segments are scanned separately so pad
magnitudes never leak into the next segment.

Window sizes are processed in pairs (k, 2k): S_k by prefix diff at
width 256+k, S_2k = S_k + shift(S_k), with the downstream ln / exp /
mul / sub stages batched over the [tensor, pair] dims. S2, 1/S1, D,
dD, dU, dx are bf16 (DVE 2x mode where both inputs are 16-bit); the
prefix scans, S1 and U = ln(S1) stay fp32. End-to-end loss error vs
the fp64 reference is ~3e-5. ACT runs exp/ln/|.|+accumulate from the
single natural_log_exp_and_others table set (see _patch_act_tables).
"""

import sys

import numpy as np

if "/opt/trn_rl_repo" not in sys.path:
    sys.path.insert(0, "/opt/trn_rl_repo")

import concourse.bacc as bacc
import concourse.bass as bass
import concourse.tile as tile
from concourse import mybir

KERNELS = (4, 8, 16, 32, 64, 128)
B, C, L = 64, 2, 2048
N_CORES = 8
ROWS = (B // N_CORES) * C          # 16 rows per core
CH = 8                             # chunks per row
W = L // CH                        # 256 window starts per chunk
HALO = 127                         # max k - 1
DSEG = W + HALO                    # 383 data cols per segment
SEG = DSEG + 1                     # 384 = zero col + data
PAD = 30.0                         # e^30 ~ 1.07e13 dominates real sums

F32 = mybir.dt.float32
BF16 = mybir.dt.bfloat16
AF = mybir.ActivationFunctionType
OP = mybir.AluOpType

_CACHE: dict = {}
SCAN_ON_GPSIMD = False


def _patch_act_tables():
    """Keep Exp/Ln/Abs resolvable only via natural_log_exp_and_others so
    the table-load pass emits one ACT table set instead of thrashing
    between exp_and_others and natural_log (~2.7us per reload)."""
    if _CACHE.get("act_patched"):
        return
    orig = bacc.get_activation_tables
    funcs = {AF.Exp, AF.Ln, AF.Abs}

    def patched(arch):
        tables = dict(orig(arch))
        return {
            name: (fs if name == "natural_log_exp_and_others" else fs - funcs)
            for name, fs in tables.items()
        }

    bacc.get_activation_tables = patched
    _CACHE["act_patched"] = True


def build(reps: int = 1, loop_iters: int = 0):
    """reps>1 unrolls the compute body; loop_iters>0 wraps it in a HW
    For_i loop (both for timing only)."""
    _patch_act_tables()
    nc = bacc.Bacc("TRN2", target_bir_lowering=False)

    x_d = nc.dram_tensor("x", [ROWS, L], F32, kind="ExternalInput")
    t_d = nc.dram_tensor("t", [ROWS, L], F32, kind="ExternalInput")
    acc_d = nc.dram_tensor("acc", [128, len(KERNELS)], F32, kind="ExternalOutput")

    with tile.TileContext(nc) as tc:
        with (
            tc.tile_pool(name="big", bufs=1) as big,
            tc.tile_pool(name="work", bufs=3) as work,
        ):
            X = big.tile([128, 2 * DSEG], F32)      # [x-data | t-data]
            EX = big.tile([128, 4 * SEG], F32)      # 4 x [0 | data]
            P = big.tile([128, 2048], F32)          # prefix sums (padded)
            ACC = big.tile([128, len(KERNELS)], F32)
            PADT = big.tile([16, HALO], F32)        # pad source

            # ---- load x and t into the halo layout ----
            nc.gpsimd.memset(PADT[:, :], PAD)
            for seg, dram in ((0, x_d), (1, t_d)):
                c0 = seg * DSEG
                # chunks j=0..6 -> partitions [0,112): one DMA, 383 cols
                src_main = bass.AP(
                    tensor=dram[:, :].tensor,
                    offset=0,
                    ap=[[W, CH - 1], [L, ROWS], [1, DSEG]],
                )
                nc.sync.dma_start(out=X[0:112, c0 : c0 + DSEG], in_=src_main)
                # chunk j=7 -> partitions [112,128): 256 real cols + pad tail
                nc.sync.dma_start(
                    out=X[112:128, c0 : c0 + W], in_=dram[0:ROWS, L - W : L]
                )
                nc.sync.dma_start(
                    out=X[112:128, c0 + W : c0 + DSEG], in_=PADT[:, :]
                )

            # zero col at the head of each scan segment
            EX4 = EX[:, :].rearrange("p (s f) -> p s f", s=4)
            nc.vector.memset(EX4[:, :, 0:1], 0.0)

            def compute_body():
                # ---- E = exp(X), XE = X * E; split by tensor so the x
                # pipeline overlaps the t DMA ----
                for a in range(2):
                    xa = X[:, a * DSEG : (a + 1) * DSEG]
                    e_sl = EX[:, 2 * a * SEG + 1 : (2 * a + 1) * SEG]
                    xe_sl = EX[:, (2 * a + 1) * SEG + 1 : (2 * a + 2) * SEG]
                    nc.scalar.activation(out=e_sl, in_=xa, func=AF.Exp)
                    nc.vector.tensor_tensor(out=xe_sl, in0=xa, in1=e_sl, op=OP.mult)
                    for s in (2 * a, 2 * a + 1):
                        scan_eng = nc.gpsimd if SCAN_ON_GPSIMD else nc.vector
                        scan_eng.tensor_tensor_scan(
                            out=P[:, s * SEG : (s + 1) * SEG],
                            data0=EX[:, s * SEG : (s + 1) * SEG],
                            data1=EX[:, s * SEG : (s + 1) * SEG],
                            initial=0.0,
                            op0=OP.add,
                            op1=OP.bypass,
                        )

                # ---- per pair of window sizes (k, 2k) ----
                # S_k via prefix diff (width 256+k); S_2k = S_k + shift(S_k).
                # S2/R/D/N/dx in bf16 (loss rel err ~1e-4); S1/U stay fp32.
                for pi, k in enumerate((4, 16, 64)):
                    kw = W + k
                    S1 = work.tile([128, 2, 2, SEG], F32)   # [a, kk, f]
                    S2 = work.tile([128, 2, 2, SEG], BF16)
                    pE_hi = P[:, k : k + 4 * SEG].rearrange(
                        "p (s f) -> p s f", s=2
                    )
                    pE_lo = P[:, 0 : 4 * SEG].rearrange("p (s f) -> p s f", s=2)
                    pX_hi = P[:, SEG + k : SEG + k + 4 * SEG].rearrange(
                        "p (s f) -> p s f", s=2
                    )
                    pX_lo = P[:, SEG : SEG + 4 * SEG].rearrange(
                        "p (s f) -> p s f", s=2
                    )
                    pE_hi2 = P[:, 2 * k : 2 * k + 4 * SEG].rearrange(
                        "p (s f) -> p s f", s=2
                    )
                    nc.vector.tensor_tensor(
                        out=S1[:, :, 0, 0:W], in0=pE_hi[:, :, 0:W],
                        in1=pE_lo[:, :, 0:W], op=OP.subtract,
                    )
                    nc.vector.tensor_tensor(
                        out=S1[:, :, 1, 0:W], in0=pE_hi2[:, :, 0:W],
                        in1=pE_lo[:, :, 0:W], op=OP.subtract,
                    )
                    nc.vector.tensor_tensor(
                        out=S2[:, :, 0, 0:kw], in0=pX_hi[:, :, 0:kw],
                        in1=pX_lo[:, :, 0:kw], op=OP.subtract,
                    )
                    nc.vector.tensor_tensor(
                        out=S2[:, :, 1, 0:W], in0=S2[:, :, 0, 0:W],
                        in1=S2[:, :, 0, k : k + W], op=OP.add,
                    )

                    U = work.tile([128, 2, 2, W], F32)    # ln S1
                    R = work.tile([128, 2, 2, W], BF16)   # 1/S1 = exp(-U)
                    D = work.tile([128, 2, 2, W], BF16)   # S2/S1
                    nc.scalar.activation(
                        out=U[:, :, :, :], in_=S1[:, :, :, 0:W], func=AF.Ln
                    )
                    nc.scalar.activation(
                        out=R[:, :, :, :], in_=U[:, :, :, :], func=AF.Exp,
                        scale=-1.0,
                    )
                    nc.vector.tensor_tensor(
                        out=D[:, :, :, :], in0=S2[:, :, :, 0:W],
                        in1=R[:, :, :, :], op=OP.mult,
                    )

                    # dx = (Dx-Dt) - (Ux-Ut): rounding dU after the
                    # subtract is cheaper (2x bf16 ops) and more accurate
                    # than materializing negentropy N = D - U.
                    dD = work.tile([128, 2, W], BF16)
                    dU = work.tile([128, 2, W], BF16)
                    dx = work.tile([128, 2, W], BF16)
                    dxa = work.tile([128, 2, W], BF16)
                    nc.vector.tensor_tensor(
                        out=dD[:, :, :], in0=D[:, 0, :, :],
                        in1=D[:, 1, :, :], op=OP.subtract,
                    )
                    nc.vector.tensor_tensor(
                        out=dU[:, :, :], in0=U[:, 0, :, :],
                        in1=U[:, 1, :, :], op=OP.subtract,
                    )
                    nc.vector.tensor_tensor(
                        out=dx[:, :, :], in0=dD[:, :, :], in1=dU[:, :, :],
                        op=OP.subtract,
                    )
                    for kki in range(2):
                        nc.scalar.activation(
                            out=dxa[:, kki, :], in_=dx[:, kki, :], func=AF.Abs,
                            accum_out=ACC[:, 2 * pi + kki : 2 * pi + kki + 1],
                        )
                    yield pi

            if loop_iters:
                with tc.For_i(0, loop_iters, 1):
                    for _ in compute_body():
                        pass
                nc.sync.dma_start(out=acc_d[:, :], in_=ACC[:, :])
            else:
                for r in range(reps):
                    for pi in compute_body():
                        if r == reps - 1:
                            # stream partial results out as pairs finish
                            nc.sync.dma_start(
                                out=acc_d[:, 2 * pi : 2 * pi + 2],
                                in_=ACC[:, 2 * pi : 2 * pi + 2],
                            )

    nc.compile()
    return nc


def make_runner(nc):
    """Once-jitted 8-core runner (run_bass_via_pjrt re-traces per call)."""
    import jax
    from jax.sharding import Mesh, PartitionSpec
    from jax.experimental.shard_map import shard_map
    from concourse import bass2jax
    from concourse import mybir as mb

    bass2jax.install_neuronx_cc_hook()

    part_name = nc.partition_id_tensor.name if nc.partition_id_tensor else None
    in_names, out_names, out_avals, zero_outs = [], [], [], []
    for alloc in nc.m.functions[0].allocations:
        if not isinstance(alloc, mb.MemoryLocationSet):
            continue
        name = alloc.memorylocations[0].name
        if alloc.kind == "ExternalInput":
            if name != part_name:
                in_names.append(name)
        elif alloc.kind == "ExternalOutput":
            shape = tuple(alloc.tensor_shape)
            dtype = mb.dt.np(alloc.dtype)
            out_names.append(name)
            out_avals.append(jax.core.ShapedArray(shape, dtype))
            zero_outs.append(np.zeros(shape, dtype))
    n_params = len(in_names)
    all_names = in_names + out_names
    if part_name is not None:
        all_names = all_names + [part_name]
    donate = tuple(range(n_params, n_params + len(out_names)))

    def _body(*args):
        operands = list(args)
        if part_name is not None:
            operands.append(bass2jax.partition_id_tensor())
        outs = bass2jax._bass_exec_p.bind(
            *operands,
            out_avals=tuple(out_avals),
            in_names=tuple(all_names),
            out_names=tuple(out_names),
            lowering_input_output_aliases=(),
            sim_require_finite=True,
            sim_require_nnan=True,
            nc=nc,
        )
        return tuple(outs)

    devices = jax.devices()[:N_CORES]
    mesh = Mesh(np.asarray(devices), ("core",))
    n_args = n_params + len(out_names)
    sharded = jax.jit(
        shard_map(
            _body,
            mesh=mesh,
            in_specs=(PartitionSpec("core"),) * n_args,
            out_specs=(PartitionSpec("core"),) * len(out_names),
            check_rep=False,
        ),
        donate_argnums=donate,
        keep_unused=True,
    )

    def run(in_maps):
        concat_in = [
            np.concatenate([np.asarray(m[name]) for m in in_maps], axis=0)
            for name in in_names
        ]
        concat_zeros = [
            np.zeros((N_CORES * z.shape[0], *z.shape[1:]), z.dtype)
            for z in zero_outs
        ]
        out_arrs = sharded(*concat_in, *concat_zeros)
        out_arrs = [np.asarray(a) for a in out_arrs]
        return [
            {
                name: out_arrs[i].reshape(N_CORES, *out_avals[i].shape)[c]
                for i, name in enumerate(out_names)
            }
            for c in range(N_CORES)
        ]

    return run


def kernel(input: np.ndarray, target: np.ndarray) -> np.ndarray:
    if "run" not in _CACHE:
        _CACHE["nc"] = build()
        _CACHE["run"] = make_runner(_CACHE["nc"])

    x = np.ascontiguousarray(input, dtype=np.float32).reshape(N_CORES, ROWS, L)
    t = np.ascontiguousarray(target, dtype=np.float32).reshape(N_CORES, ROWS, L)
    in_maps = [{"x": x[c], "t": t[c]} for c in range(N_CORES)]

    results = _CACHE["run"](in_maps)
    acc = np.stack([r["acc"] for r in results])      # [cores, 128, 6]

    per_k = acc.sum(axis=(0, 1), dtype=np.float64)   # [6]
    counts = np.array([B * C * (L - k + 1) for k in KERNELS], dtype=np.float64)
    return np.float32((per_k / counts).sum())
